# revision 5
# baseline (speedup 1.0000x reference)
"""Trainium2 Bass kernel: ConvolutionalMultiheadAttention.

Reference computation (per batch element b):
    q = conv1d(x, w0) + b0          # [D, Lp]  (VALID, K=3)
    k = conv1d(x, w1) + b1
    v = conv1d(x, w2) + b2
    per head h (Dh=64): out_h = softmax(q_h^T k_h / sqrt(D)) applied to v_h
    out[b] = concat_h(v_h @ attn^T)  # [D, Lp]

Sharding: data-parallel over batch B=16 across 8 cores (2 per core).
Weights replicated. No collectives.

Per-core kernel architecture:
  - conv as matmul: contraction over input channel i (4 chunks of 128),
    accumulating 4*3 = 12 matmuls per PSUM tile. q,k produced in
    [o_part, t_free] layout; v produced transposed [t_part, o_free]
    (lhsT = x slice, rhs = w2 slice) with a constant 1.0 column appended
    per head (65-wide) so the attention AV matmul also yields the
    softmax denominator row.
  - scores computed transposed: S_T[kt, qt] = k_h^T q_h (contraction
    over d=64 on partitions). exp via ACT engine with the 1/sqrt(512)
    scale folded into the activation, output in fp16 (P_T).
  - AV: out[65, qt] = [v_h | 1]^T @ P_T accumulated over kt chunks.
    Row 64 is the softmax denominator. Normalize: reciprocal (DVE) +
    partition_broadcast (GPSIMD) + multiply (DVE), then DMA straight to
    the output in [o, t] layout.
  - fp32r matmuls (full PE rate at N>=256) for convs/scores; fp16 for
    the P_T/V attention matmul (P in [0, e^4], fp16 rel err ~5e-4).
"""

import numpy as np

import concourse.bass as bass
import concourse.bacc as bacc
import concourse.mybir as mybir
import concourse.tile as tile
from concourse.bass_utils import run_bass_kernel_spmd

B, D, L, KW, H = 16, 512, 1024, 3, 8
LP = L - KW + 1          # 1022
DH = D // H              # 64
NCORES = 8
BLOC = B // NCORES       # 2
NIC = D // 128           # 4 input-channel chunks
SCALE = 1.0 / float(np.sqrt(D))

F32 = mybir.dt.float32
F32R = mybir.dt.float32r
F16 = mybir.dt.float16

# time chunking
TQ = [(0, 512), (512, LP - 512)]                       # qt chunks (512, 510)
TKC = [(i * 128, min(128, LP - i * 128)) for i in range(8)]  # kt chunks (...126)


def _emit(tc, xs, wq, wk, wv, bq, bk, bv, out):
    nc = tc.nc
    Exp = mybir.ActivationFunctionType.Exp
    from concourse.alu_op_type import AluOpType
    Add = AluOpType.add

    from contextlib import ExitStack
    ctx = ExitStack()
    wpool = ctx.enter_context(tc.tile_pool(name="w", bufs=1))
    cpool = ctx.enter_context(tc.tile_pool(name="const", bufs=1))
    xpool = ctx.enter_context(tc.tile_pool(name="x", bufs=1))
    qkpool = ctx.enter_context(tc.tile_pool(name="qk", bufs=1))
    vpool = ctx.enter_context(tc.tile_pool(name="v", bufs=1))
    ptpool = ctx.enter_context(tc.tile_pool(name="pt", bufs=12))
    opool = ctx.enter_context(tc.tile_pool(name="o", bufs=4))
    rpool = ctx.enter_context(tc.tile_pool(name="r", bufs=4))
    bpool = ctx.enter_context(tc.tile_pool(name="bc", bufs=4))
    pconv = ctx.enter_context(tc.tile_pool(name="pconv", bufs=3, space="PSUM"))
    pscore = ctx.enter_context(tc.tile_pool(name="pscore", bufs=3, space="PSUM"))
    pav = ctx.enter_context(tc.tile_pool(name="pav", bufs=2, space="PSUM"))

    # ---- loads ----
    w_sb = {}
    for nm, wdram in (("q", wq), ("k", wk), ("v", wv)):
        t = wpool.tile([128, NIC, KW, D], F32R, tag=f"w{nm}")
        nc.sync.dma_start(t[:], wdram[:])
        w_sb[nm] = t
    bq_sb = cpool.tile([128, NIC], F32, tag="bq")
    nc.sync.dma_start(bq_sb[:], bq[:])
    bk_sb = cpool.tile([128, NIC], F32, tag="bk")
    nc.sync.dma_start(bk_sb[:], bk[:])
    bv_sb = cpool.tile([128, D], F32, tag="bv")
    nc.sync.dma_start(bv_sb[:], bv[:])

    x_sb = xpool.tile([128, BLOC, NIC, L], F32R)
    nc.sync.dma_start(x_sb[:], xs.rearrange("b (c p) t -> p b c t", p=128))

    # q/k: [p, oc, t] with o = oc*128+p ; v: [p(t in chunk), ktc, h, 0:64]+ones
    q_sb = qkpool.tile([128, NIC, L], F32R, tag="q")
    k_sb = qkpool.tile([128, NIC, L], F32R, tag="k")
    v_sb = vpool.tile([128, 8, H, DH + 1], F16, tag="v")
    nc.gpsimd.memset(v_sb[:, :, :, DH:DH + 1], 1.0)

    for b in range(BLOC):
        # ---- convs ----
        # q, k in [o, t] layout
        for nm, bias_sb, dst in (("q", bq_sb, q_sb), ("k", bk_sb, k_sb)):
            for oc in range(NIC):
                for (t0, tn) in TQ:
                    ps = pconv.tile([128, 512], F32, tag="pc")
                    mm = 0
                    for ic in range(NIC):
                        for kk in range(KW):
                            nc.tensor.matmul(
                                ps[:, :tn],
                                w_sb[nm][:, ic, kk, oc * 128:(oc + 1) * 128],
                                x_sb[:, b, ic, t0 + kk:t0 + kk + tn],
                                start=(mm == 0), stop=(mm == NIC * KW - 1),
                            )
                            mm += 1
                    nc.vector.tensor_scalar_add(
                        dst[:, oc, t0:t0 + tn], ps[:, :tn], bias_sb[:, oc:oc + 1]
                    )
        # v transposed: [t, o] (+bias broadcast along free dim)
        for tci, (t0, tn) in enumerate(TKC):
            ps = pconv.tile([128, 512], F32, tag="pc")
            mm = 0
            for ic in range(NIC):
                for kk in range(KW):
                    nc.tensor.matmul(
                        ps[:tn, :],
                        x_sb[:, b, ic, t0 + kk:t0 + kk + tn],
                        w_sb["v"][:, ic, kk, :],
                        start=(mm == 0), stop=(mm == NIC * KW - 1),
                    )
                    mm += 1
            nc.vector.tensor_tensor(
                v_sb[:tn, tci, :, 0:DH],
                ps[:tn].rearrange("p (h d) -> p h d", h=H),
                bv_sb[:tn].rearrange("p (h d) -> p h d", h=H),
                op=Add,
            )

        # ---- attention, software-pipelined: scores(h+1) before AV(h) ----
        pt_tiles = {}

        def scores(h):
            po = 64 * (h % 2)
            oc = h // 2
            kh = k_sb[po:po + DH, oc, :]
            qh = q_sb[po:po + DH, oc, :]
            tiles = []
            for (kt0, ktn) in TKC:
                pt = ptpool.tile([128, L], F16, tag="pt")
                for (qt0, qtn) in TQ:
                    ss = pscore.tile([128, 512], F32, tag="ps")
                    nc.tensor.matmul(
                        ss[:ktn, :qtn],
                        kh[:, kt0:kt0 + ktn],
                        qh[:, qt0:qt0 + qtn],
                        start=True, stop=True,
                    )
                    nc.scalar.activation(
                        pt[:ktn, qt0:qt0 + qtn], ss[:ktn, :qtn], Exp, scale=SCALE
                    )
                tiles.append(pt)
            pt_tiles[h] = tiles

        def av(h):
            tiles = pt_tiles.pop(h)
            for (qt0, qtn) in TQ:
                pa = pav.tile([128, 512], F32, tag="pa")
                for tci, (kt0, ktn) in enumerate(TKC):
                    nc.tensor.matmul(
                        pa[:DH + 1, :qtn],
                        v_sb[:ktn, tci, h, :],
                        tiles[tci][:ktn, qt0:qt0 + qtn],
                        start=(tci == 0), stop=(tci == len(TKC) - 1),
                    )
                rec = rpool.tile([1, 512], F32, tag="rec")
                nc.vector.reciprocal(rec[:1, :qtn], pa[DH:DH + 1, :qtn])
                brd = bpool.tile([DH, 512], F32, tag="brd")
                nc.gpsimd.partition_broadcast(brd[:, :qtn], rec[:1, :qtn])
                ot = opool.tile([DH, 512], F32, tag="ot")
                nc.vector.tensor_mul(ot[:, :qtn], pa[0:DH, :qtn], brd[:, :qtn])
                nc.sync.dma_start(
                    out[b, DH * h:DH * (h + 1), qt0:qt0 + qtn], ot[:, :qtn]
                )

        scores(0)
        for h in range(H):
            if h + 1 < H:
                scores(h + 1)
            av(h)

    ctx.close()


_CACHE = {}


def _build():
    if "nc" in _CACHE:
        return _CACHE["nc"]
    nc = bacc.Bacc("TRN2", target_bir_lowering=False, debug=False,
                   num_devices=NCORES)
    xs = nc.dram_tensor("xs", [BLOC, D, L], F32R, kind="ExternalInput").ap()
    wq = nc.dram_tensor("wqt", [128, NIC, KW, D], F32R, kind="ExternalInput").ap()
    wk = nc.dram_tensor("wkt", [128, NIC, KW, D], F32R, kind="ExternalInput").ap()
    wv = nc.dram_tensor("wvt", [128, NIC, KW, D], F32R, kind="ExternalInput").ap()
    bq = nc.dram_tensor("bq", [128, NIC], F32, kind="ExternalInput").ap()
    bk = nc.dram_tensor("bk", [128, NIC], F32, kind="ExternalInput").ap()
    bv = nc.dram_tensor("bv", [128, D], F32, kind="ExternalInput").ap()
    out = nc.dram_tensor("out", [BLOC, D, LP], F32, kind="ExternalOutput").ap()
    with tile.TileContext(nc) as tc:
        _emit(tc, xs, wq, wk, wv, bq, bk, bv, out)
    nc.compile()
    _CACHE["nc"] = nc
    return nc


def _wt(w):
    # w: [O, I, K] -> [p, ic, k, o] with i = ic*128 + p
    return np.ascontiguousarray(
        w.transpose(1, 2, 0).reshape(NIC, 128, KW, D).transpose(1, 0, 2, 3)
    ).astype(np.float32)


def kernel(x, w0, b0, w1, b1, w2, b2):
    x = np.asarray(x, dtype=np.float32)
    inp_common = {
        "wqt": _wt(np.asarray(w0, np.float32)),
        "wkt": _wt(np.asarray(w1, np.float32)),
        "wvt": _wt(np.asarray(w2, np.float32)),
        # per-partition bias layouts: [p, oc] with o = oc*128+p
        "bq": np.ascontiguousarray(
            np.asarray(b0, np.float32).reshape(NIC, 128).T),
        "bk": np.ascontiguousarray(
            np.asarray(b1, np.float32).reshape(NIC, 128).T),
        "bv": np.ascontiguousarray(
            np.tile(np.asarray(b2, np.float32)[None, :], (128, 1))),
    }
    nc = _build()
    in_maps = [
        {"xs": np.ascontiguousarray(x[c * BLOC:(c + 1) * BLOC]), **inp_common}
        for c in range(NCORES)
    ]
    res = run_bass_kernel_spmd(nc, in_maps, list(range(NCORES)))
    return np.concatenate([res.results[c]["out"] for c in range(NCORES)], axis=0)


def run_traced(x, w0, b0, w1, b1, w2, b2, **kw):
    """Like kernel() but returns (output, BassKernelResults) with trace."""
    x = np.asarray(x, dtype=np.float32)
    inp_common = {
        "wqt": _wt(np.asarray(w0, np.float32)),
        "wkt": _wt(np.asarray(w1, np.float32)),
        "wvt": _wt(np.asarray(w2, np.float32)),
        "bq": np.ascontiguousarray(
            np.asarray(b0, np.float32).reshape(NIC, 128).T),
        "bk": np.ascontiguousarray(
            np.asarray(b1, np.float32).reshape(NIC, 128).T),
        "bv": np.ascontiguousarray(
            np.tile(np.asarray(b2, np.float32)[None, :], (128, 1))),
    }
    nc = _build()
    in_maps = [
        {"xs": np.ascontiguousarray(x[c * BLOC:(c + 1) * BLOC]), **inp_common}
        for c in range(NCORES)
    ]
    res = run_bass_kernel_spmd(nc, in_maps, list(range(NCORES)), **kw)
    out = np.concatenate([res.results[c]["out"] for c in range(NCORES)], axis=0)
    return out, res


# revision 29
# speedup vs baseline: 122.7582x; 122.7582x over previous
"""Trainium2 Bass kernel: ConvolutionalMultiheadAttention.

Reference computation (per batch element b):
    q = conv1d(x, w0) + b0          # [D, Lp]  (VALID, K=3)
    k = conv1d(x, w1) + b1
    v = conv1d(x, w2) + b2
    per head h (Dh=64): out_h = v_h @ softmax(q_h^T k_h / sqrt(D))^T

Sharding: data-parallel over batch B=16 across 8 cores (2 per core).
Weights replicated. No collectives.

Per-core kernel architecture:
  - conv as matmul: contraction over input channel i (4 chunks of 128),
    accumulating 4*3 = 12 matmuls per PSUM tile. q,k produced in
    [o_part, t_free] layout; v produced transposed [t_part, o_free]
    (lhsT = x slice, rhs = w2 slice) with a constant 1.0 column appended
    per head (65-wide) so the attention AV matmul also yields the
    softmax denominator row.
  - scores computed transposed: S_T[kt, qt] = k_h^T q_h (contraction
    over d=64 on partitions). exp via ACT engine with the 1/sqrt(512)
    scale folded into the activation, output in fp16 (P_T).
  - AV: out[65, qt] = [v_h | 1]^T @ P_T accumulated over kt chunks.
    Row 64 is the softmax denominator. Normalize: reciprocal (DVE) +
    partition_broadcast (GPSIMD) + multiply (DVE), then DMA straight to
    the output in [o, t] layout.
  - fp32r matmuls (full PE rate at N>=256) for convs/scores; fp16 for
    the P_T/V attention matmul (P in [0, e^4], fp16 rel err ~5e-4).
  - input DMAs split per chunk so the first conv matmuls start ~3.5us
    in; conv of batch b=1 is interleaved into the attention pair loop
    of b=0 so the PE has work while ACT streams exps.
"""

import numpy as np

import concourse.bass as bass
import concourse.bacc as bacc
import concourse.mybir as mybir
import concourse.tile as tile
from concourse.bass_utils import run_bass_kernel_spmd

B, D, L, KW, H = 16, 512, 1024, 3, 8
LP = L - KW + 1          # 1022
DH = D // H              # 64
NCORES = 8
BLOC = B // NCORES       # 2
NIC = D // 128           # 4 input-channel chunks
SCALE = 1.0 / float(np.sqrt(D))
IC_MAJOR = False

F32 = mybir.dt.float32
F32R = mybir.dt.float32r
F16 = mybir.dt.float16

# time chunking
TQ = [(0, 512), (512, LP - 512)]                       # qt chunks (512, 510)
TKC = [(i * 128, min(128, LP - i * 128)) for i in range(8)]  # kt chunks (...126)


def _emit(tc, xs, wq, wk, wv, bq, bk, bv, out, loop_n=None):
    nc = tc.nc
    Exp = mybir.ActivationFunctionType.Exp
    from concourse.alu_op_type import AluOpType
    Add = AluOpType.add
    from contextlib import ExitStack
    ctx = ExitStack()
    wpool = ctx.enter_context(tc.tile_pool(name="w", bufs=1))
    cpool = ctx.enter_context(tc.tile_pool(name="const", bufs=1))
    xpool = ctx.enter_context(tc.tile_pool(name="x", bufs=1))
    qkpool = ctx.enter_context(tc.tile_pool(name="qk", bufs=1))
    vpool = ctx.enter_context(tc.tile_pool(name="v", bufs=2))
    ptpool = ctx.enter_context(tc.tile_pool(name="pt", bufs=12))
    opool = ctx.enter_context(tc.tile_pool(name="o", bufs=3))
    rpool = ctx.enter_context(tc.tile_pool(name="r", bufs=2))
    bpool = ctx.enter_context(tc.tile_pool(name="bc", bufs=2))
    # PSUM pools are opened in two phases (static 8-bank budget):
    # phase 1: pconv8 (8 banks, b0 convs) — closed before phase 2
    # phase 2: pconv (2) + pscore (2x2) + pav (2)
    # Under loop_n (HW-timing loop), a single phase is used so no pool
    # opens/closes inside the For_i body.
    psum_pools = {}
    two_phase = IC_MAJOR and loop_n is None
    if not two_phase:
        psum_pools["pconv"] = ctx.enter_context(
            tc.tile_pool(name="pconv", bufs=2, space="PSUM"))
        psum_pools["pscore"] = ctx.enter_context(
            tc.tile_pool(name="pscore", bufs=2, space="PSUM"))
        psum_pools["pav"] = ctx.enter_context(
            tc.tile_pool(name="pav", bufs=2, space="PSUM"))

    if loop_n is not None:
        loop_cm = tc.For_i(0, loop_n, 1)
        loop_cm.__enter__()

    # ---- loads (split + ordered so the first conv matmuls start early
    # and each conv's weights land just before it needs them) ----
    wq_ic = []
    wk_ic = []
    x_t = [[None] * NIC for _ in range(BLOC)]
    for ic in range(NIC):
        t = wpool.tile([128, KW, D], F32R, tag=f"wq{ic}", name=f"wq{ic}")
        nc.sync.dma_start(t[:], wq[:, ic])
        wq_ic.append(t)
        xt = xpool.tile([128, L], F32R, tag=f"x0{ic}", name=f"x0{ic}")
        nc.sync.dma_start(
            xt[:], xs[0].rearrange("(c p) t -> p c t", p=128)[:, ic])
        x_t[0][ic] = xt
        if ic == 0:
            bq_sb = cpool.tile([128, NIC], F32, tag="bq")
            nc.sync.dma_start(bq_sb[:], bq[:])
            bk_sb = cpool.tile([128, NIC], F32, tag="bk")
            nc.sync.dma_start(bk_sb[:], bk[:])
            bv_sb = cpool.tile([128, D], F32, tag="bv")
            nc.sync.dma_start(bv_sb[:], bv[:])
    for ic in range(NIC):
        t = wpool.tile([128, KW, D], F32R, tag=f"wk{ic}", name=f"wk{ic}")
        nc.sync.dma_start(t[:], wk[:, ic])
        wk_ic.append(t)
    wv_sb = wpool.tile([128, NIC, KW, D], F32R, tag="wv")
    nc.sync.dma_start(wv_sb[:], wv[:])
    for ic in range(NIC):
        xt = xpool.tile([128, L], F32R, tag=f"x1{ic}", name=f"x1{ic}")
        nc.sync.dma_start(
            xt[:], xs[1].rearrange("(c p) t -> p c t", p=128)[:, ic])
        x_t[1][ic] = xt

    def w_slice(nm, ic, kk, osl):
        if nm == "q":
            return wq_ic[ic][:, kk, osl]
        if nm == "k":
            return wk_ic[ic][:, kk, osl]
        return wv_sb[:, ic, kk, osl]

    # q/k: per-oc tiles [p, t] with o = oc*128+p (reused in-place across b)
    q_oc = [qkpool.tile([128, L], F32R, tag=f"q{oc}", name=f"q{oc}") for oc in range(NIC)]
    k_oc = [qkpool.tile([128, L], F32R, tag=f"k{oc}", name=f"k{oc}") for oc in range(NIC)]
    # v: [p(t in chunk), ktc, h, 0:64] + ones col; double-buffered across b
    v_tiles = [None, None]

    def conv_qk_piece(b, nm, oc):
        dst = (q_oc if nm == "q" else k_oc)[oc]
        bias_sb = bq_sb if nm == "q" else bk_sb
        for (t0, tn) in TQ:
            ps = psum_pools["pconv"].tile([128, 512], F32, tag="pc", name="pc")
            mm = 0
            for ic in range(NIC):
                for kk in range(KW):
                    nc.tensor.matmul(
                        ps[:, :tn],
                        w_slice(nm, ic, kk, slice(oc * 128, (oc + 1) * 128)),
                        x_t[b][ic][:, t0 + kk:t0 + kk + tn],
                        start=(mm == 0), stop=(mm == NIC * KW - 1),
                    )
                    mm += 1
            nc.vector.tensor_tensor(
                dst[:, t0:t0 + tn], ps[:, :tn],
                bias_sb[:, oc:oc + 1].broadcast_to([128, tn]), op=Add,
            )

    def v_alloc(b):
        v_sb = vpool.tile([128, 8, H, DH + 1], F16, tag="v")
        nc.gpsimd.memset(v_sb[:, :, :, DH:DH + 1], 1.0)
        v_tiles[b] = v_sb

    def conv_v_piece(b, tci):
        t0, tn = TKC[tci]
        ps = psum_pools["pconv"].tile([128, 512], F32, tag="pc", name="pc")
        mm = 0
        for ic in range(NIC):
            for kk in range(KW):
                nc.tensor.matmul(
                    ps[:tn, :],
                    x_t[b][ic][:, t0 + kk:t0 + kk + tn],
                    wv_sb[:, ic, kk, :],
                    start=(mm == 0), stop=(mm == NIC * KW - 1),
                )
                mm += 1
        nc.vector.tensor_tensor(
            v_tiles[b][:tn, tci, :, 0:DH],
            ps[:tn].rearrange("p (h d) -> p h d", h=H),
            bv_sb[:tn].rearrange("p (h d) -> p h d", h=H),
            op=Add,
        )

    # ---- attention ----
    pt_tiles = {}

    def scores(b, h):
        po = 64 * (h % 2)
        oc = h // 2
        kh = k_oc[oc][po:po + DH, :]
        qh = q_oc[oc][po:po + DH, :]
        tiles = []
        for (kt0, ktn) in TKC:
            pt = ptpool.tile([128, L], F16, tag="pt")
            ss = psum_pools["pscore"].tile([128, 1024], F32, tag="ps", name="ss")
            for (qt0, qtn) in TQ:
                nc.tensor.matmul(
                    ss[:ktn, qt0:qt0 + qtn],
                    kh[:, kt0:kt0 + ktn],
                    qh[:, qt0:qt0 + qtn],
                    start=True, stop=True,
                )
            nc.scalar.activation(pt[:ktn, 0:LP], ss[:ktn, 0:LP], Exp,
                                 scale=SCALE)
            tiles.append(pt)
        pt_tiles[(b, h)] = tiles

    def av(b, h):
        tiles = pt_tiles.pop((b, h))
        for (qt0, qtn) in TQ:
            pa = psum_pools["pav"].tile([128, 512], F32, tag="pa", name="pa")
            for tci, (kt0, ktn) in enumerate(TKC):
                nc.tensor.matmul(
                    pa[:DH + 1, :qtn],
                    v_tiles[b][:ktn, tci, h, :],
                    tiles[tci][:ktn, qt0:qt0 + qtn],
                    start=(tci == 0), stop=(tci == len(TKC) - 1),
                )
            rec = rpool.tile([1, 512], F32, tag="rec")
            nc.vector.reciprocal(rec[:1, :qtn], pa[DH:DH + 1, :qtn])
            brd = bpool.tile([DH, 512], F32, tag="brd")
            nc.gpsimd.partition_broadcast(brd[:, :qtn], rec[:1, :qtn])
            ot = opool.tile([DH, 512], F32, tag="ot")
            nc.vector.tensor_mul(ot[:, :qtn], pa[0:DH, :qtn], brd[:, :qtn])
            nc.sync.dma_start(
                out[b, DH * h:DH * (h + 1), qt0:qt0 + qtn], ot[:, :qtn]
            )

    # conv b=0: with an 8-bank scoped PSUM pool (closed before the
    # attention PSUM pools open — PSUM pools reserve banks statically).
    def conv_b0_with_pool(pconv8):

        def conv_qk_b0_icmajor(nm):
            dst_l = q_oc if nm == "q" else k_oc
            bias_sb = bq_sb if nm == "q" else bk_sb
            groups = [(oc, t0, tn) for oc in range(NIC) for (t0, tn) in TQ]
            tiles = [pconv8.tile([128, 512], F32, tag="pc8",
                                 name=f"pc8_{nm}{gi}")
                     for gi in range(len(groups))]
            for ic in range(NIC):
                for kk in range(KW):
                    for gi, (oc, t0, tn) in enumerate(groups):
                        nc.tensor.matmul(
                            tiles[gi][:, :tn],
                            w_slice(nm, ic, kk, slice(oc * 128, (oc + 1) * 128)),
                            x_t[0][ic][:, t0 + kk:t0 + kk + tn],
                            start=(ic == 0 and kk == 0),
                            stop=(ic == NIC - 1 and kk == KW - 1),
                        )
            for gi, (oc, t0, tn) in enumerate(groups):
                nc.vector.tensor_tensor(
                    dst_l[oc][:, t0:t0 + tn], tiles[gi][:, :tn],
                    bias_sb[:, oc:oc + 1].broadcast_to([128, tn]), op=Add,
                )

        def conv_v_b0_icmajor():
            tiles = [pconv8.tile([128, 512], F32, tag="pc8",
                                 name=f"pc8_v{gi}")
                     for gi in range(len(TKC))]
            for ic in range(NIC):
                for kk in range(KW):
                    for gi, (t0, tn) in enumerate(TKC):
                        nc.tensor.matmul(
                            tiles[gi][:tn, :],
                            x_t[0][ic][:, t0 + kk:t0 + kk + tn],
                            wv_sb[:, ic, kk, :],
                            start=(ic == 0 and kk == 0),
                            stop=(ic == NIC - 1 and kk == KW - 1),
                        )
            for gi, (t0, tn) in enumerate(TKC):
                nc.vector.tensor_tensor(
                    v_tiles[0][:tn, gi, :, 0:DH],
                    tiles[gi][:tn].rearrange("p (h d) -> p h d", h=H),
                    bv_sb[:tn].rearrange("p (h d) -> p h d", h=H),
                    op=Add,
                )

        if IC_MAJOR:
            conv_qk_b0_icmajor("q")
            conv_qk_b0_icmajor("k")
            v_alloc(0)
            conv_v_b0_icmajor()
        else:
            groups = [(oc, t0, tn) for oc in range(NIC) for (t0, tn) in TQ]
            for nm in ("q", "k"):
                dst_l = q_oc if nm == "q" else k_oc
                bias_sb = bq_sb if nm == "q" else bk_sb
                for (oc, t0, tn) in groups:
                    ps = pconv8.tile([128, 512], F32, tag="pc8", name="pc8")
                    mm = 0
                    for ic in range(NIC):
                        for kk in range(KW):
                            nc.tensor.matmul(
                                ps[:, :tn],
                                w_slice(nm, ic, kk,
                                        slice(oc * 128, (oc + 1) * 128)),
                                x_t[0][ic][:, t0 + kk:t0 + kk + tn],
                                start=(mm == 0), stop=(mm == NIC * KW - 1),
                            )
                            mm += 1
                    nc.vector.tensor_tensor(
                        dst_l[oc][:, t0:t0 + tn], ps[:, :tn],
                        bias_sb[:, oc:oc + 1].broadcast_to([128, tn]), op=Add,
                    )
            v_alloc(0)
            for gi, (t0, tn) in enumerate(TKC):
                ps = pconv8.tile([128, 512], F32, tag="pc8", name="pc8")
                mm = 0
                for ic in range(NIC):
                    for kk in range(KW):
                        nc.tensor.matmul(
                            ps[:tn, :],
                            x_t[0][ic][:, t0 + kk:t0 + kk + tn],
                            wv_sb[:, ic, kk, :],
                            start=(mm == 0), stop=(mm == NIC * KW - 1),
                        )
                        mm += 1
                nc.vector.tensor_tensor(
                    v_tiles[0][:tn, gi, :, 0:DH],
                    ps[:tn].rearrange("p (h d) -> p h d", h=H),
                    bv_sb[:tn].rearrange("p (h d) -> p h d", h=H),
                    op=Add,
                )

    if two_phase:
        with tc.tile_pool(name="pconv8", bufs=8, space="PSUM") as pconv8:
            conv_b0_with_pool(pconv8)
        psum_pools["pconv"] = ctx.enter_context(
            tc.tile_pool(name="pconv", bufs=2, space="PSUM"))
        psum_pools["pscore"] = ctx.enter_context(
            tc.tile_pool(name="pscore", bufs=2, space="PSUM"))
        psum_pools["pav"] = ctx.enter_context(
            tc.tile_pool(name="pav", bufs=2, space="PSUM"))
    else:
        for oc in range(NIC):
            conv_qk_piece(0, "q", oc)
        for oc in range(NIC):
            conv_qk_piece(0, "k", oc)
        v_alloc(0)
        for tci in range(len(TKC)):
            conv_v_piece(0, tci)

    # attention b=0 with conv b=1 injected between pairs (fills PE while
    # the ACT engine streams exps; evictions wait on b=0 reads per-tile)
    def inject(h):
        if h == 0:
            v_alloc(1)
            for tci in range(4):
                conv_v_piece(1, tci)
        elif h == 1:
            for tci in range(4, 8):
                conv_v_piece(1, tci)
        elif h in (2, 3, 4):
            # q_oc[oc]/k_oc[oc] are read by scores(0, 2oc) and scores(0, 2oc+1);
            # scores(0, j) is emitted at h = j-1, so conv(1, oc) may only be
            # emitted at h >= 2oc (oc=2 lands exactly at its boundary).
            oc = h - 2
            conv_qk_piece(1, "q", oc)
            conv_qk_piece(1, "k", oc)
        elif h == 5:
            # pull b1's first score pair forward so ACT has exp work
            # queued before the conv filler runs out
            scores(1, 0)
        elif h == 6:
            conv_qk_piece(1, "q", 3)
            conv_qk_piece(1, "k", 3)
            scores(1, 1)

    scores(0, 0)
    for h in range(H):
        if h + 1 < H:
            scores(0, h + 1)
        av(0, h)
        inject(h)

    for h in range(H):
        if h + 1 < H and (1, h + 1) not in pt_tiles:
            scores(1, h + 1)
        av(1, h)

    if loop_n is not None:
        loop_cm.__exit__(None, None, None)
    ctx.close()


_CACHE = {}


def _build(loop_n=None):
    key = ("nc", loop_n)
    if key in _CACHE:
        return _CACHE[key]
    nc = bacc.Bacc("TRN2", target_bir_lowering=False, debug=False,
                   num_devices=NCORES)
    xs = nc.dram_tensor("xs", [BLOC, D, L], F32R, kind="ExternalInput").ap()
    wq = nc.dram_tensor("wqt", [128, NIC, KW, D], F32R, kind="ExternalInput").ap()
    wk = nc.dram_tensor("wkt", [128, NIC, KW, D], F32R, kind="ExternalInput").ap()
    wv = nc.dram_tensor("wvt", [128, NIC, KW, D], F32R, kind="ExternalInput").ap()
    bq = nc.dram_tensor("bq", [128, NIC], F32, kind="ExternalInput").ap()
    bk = nc.dram_tensor("bk", [128, NIC], F32, kind="ExternalInput").ap()
    bv = nc.dram_tensor("bv", [128, D], F32, kind="ExternalInput").ap()
    out = nc.dram_tensor("out", [BLOC, D, LP], F32, kind="ExternalOutput").ap()
    with tile.TileContext(nc) as tc:
        _emit(tc, xs, wq, wk, wv, bq, bk, bv, out, loop_n=loop_n)
    nc.compile()
    _CACHE[key] = nc
    return nc


def _wt(w):
    # w: [O, I, K] -> [p, ic, k, o] with i = ic*128 + p
    return np.ascontiguousarray(
        w.transpose(1, 2, 0).reshape(NIC, 128, KW, D).transpose(1, 0, 2, 3)
    ).astype(np.float32)


def _in_common(w0, b0, w1, b1, w2, b2):
    return {
        "wqt": _wt(np.asarray(w0, np.float32)),
        "wkt": _wt(np.asarray(w1, np.float32)),
        "wvt": _wt(np.asarray(w2, np.float32)),
        # per-partition bias layouts: [p, oc] with o = oc*128+p
        "bq": np.ascontiguousarray(
            np.asarray(b0, np.float32).reshape(NIC, 128).T),
        "bk": np.ascontiguousarray(
            np.asarray(b1, np.float32).reshape(NIC, 128).T),
        "bv": np.ascontiguousarray(
            np.tile(np.asarray(b2, np.float32)[None, :], (128, 1))),
    }


def kernel(x, w0, b0, w1, b1, w2, b2):
    x = np.asarray(x, dtype=np.float32)
    inp_common = _in_common(w0, b0, w1, b1, w2, b2)
    nc = _build()
    in_maps = [
        {"xs": np.ascontiguousarray(x[c * BLOC:(c + 1) * BLOC]), **inp_common}
        for c in range(NCORES)
    ]
    res = run_bass_kernel_spmd(nc, in_maps, list(range(NCORES)))
    return np.concatenate([res.results[c]["out"] for c in range(NCORES)], axis=0)


def run_traced(x, w0, b0, w1, b1, w2, b2, **kw):
    """Like kernel() but returns (output, BassKernelResults)."""
    x = np.asarray(x, dtype=np.float32)
    inp_common = _in_common(w0, b0, w1, b1, w2, b2)
    nc = _build()
    in_maps = [
        {"xs": np.ascontiguousarray(x[c * BLOC:(c + 1) * BLOC]), **inp_common}
        for c in range(NCORES)
    ]
    res = run_bass_kernel_spmd(nc, in_maps, list(range(NCORES)), **kw)
    out = np.concatenate([res.results[c]["out"] for c in range(NCORES)], axis=0)
    return out, res
